# revision 24
# baseline (speedup 1.0000x reference)
"""Bass/Trainium2 kernel for per-chunk fake-quant + linear.

reference semantics (per chunk c):
    q  = clip(round(x/s_c), -128, 127) * s_c
    out[c] = q @ w[c].T          # [B,S,O]

Strategy (16-bit I/O halves HBM traffic to ~32.5MB/core; measured ~97us
on HW vs 207-229us for the f32-I/O baseline; DMA saturates ~420 GB/s
wall-to-wall mid-kernel, which is the pole):
  - Data-parallel over tokens: each of 8 cores gets T = B*S/8 = 8192 tokens
    (all 4 chunks), weights replicated.
  - x is staged host-side as f16 [C, D, T] (transposed so the contraction
    dim d sits on SBUF partitions). f16 keeps 11 mantissa bits: the induced
    quant-decision flips add ~0.2% rel error vs the 2e-2 tolerance.
  - Device quantization in two DVE tensor_scalar passes (16-bit dtypes ->
    4x perf mode, ~0.3ns/elt):
      t16 = rne(x * 1/s)   as int16 (HW convert-on-write rounds RNE,
                            matching jnp.round; values in +-600)
      qi  = clip(t16, -128, 127) as f16 (small integers, exact in f16)
  - Weight-stationary matmuls: lhsT = ws16[c,dk,og] (128d x 128o) loaded
    once per 4 MMs, rhs = qi (128d x 512t), accumulated over dk into a
    PSUM tile [128o, 1024t] spanning 2 banks.  ws16 = (s*w).T * 2^10 as
    f16 (pre-scale keeps all weights f16-normal), the 2^-10 dequant is
    folded into the PSUM->SBUF copy scale.
  - PSUM->SBUF copies at FD=1024 (PSUM reads are 1x rate; 2-bank reads
    halve the per-op overhead), split 3 ACT : 1 DVE to balance engines
    (~55us busy each, under the ~80us DMA pole).
  - Output staged f16 [128o, (og t)] and DMA'd out as f16, one 1MB SWDGE
    store per tile-group (big stores keep the drain-phase store rate
    high); the last chunk's stores go out on the by-then-idle sync +
    scalar HWDGE rings to shorten the tail. Host upcasts to f32.
  - DMA rings: x loads on sync HWDGE (1MB tiles, 4KB runs/partition),
    weights + stores on gpsimd SWDGE.
  - Post-passes on the BIR: redundant back-to-back LDWEIGHTS with
    identical source APs are replaced by sync-preserving NOPs (weights
    stay resident in the PE array across the dk-inner MM pairs); excess
    per-instruction sem waits hoisted onto NOPs (walrus rejects >2).
"""

import numpy as np

import concourse.bass as bass
import concourse.tile as tile
import concourse.mybir as mybir
from concourse.bass_utils import run_bass_kernel_spmd


def _split_sync_waits(nc):
    """Hoist excess per-instruction sem waits onto preceding same-engine NOPs.

    This walrus build rejects instructions carrying >2 sync waits ("Too many
    sync wait commands", CoreV2/V3GenImpl setupSyncWait). A NOP on the same
    engine immediately before the instruction blocks the queue identically,
    so semantics are preserved.
    """
    count = 0
    for fn in nc.m.functions:
        for bb in fn.blocks:
            out = []
            for ins in bb.instructions:
                si = ins.sync_info
                waits = list(si.on_wait) if (si and si.on_wait) else []
                maxw = 1
                if len(waits) > maxw:
                    extra, keep = waits[:-maxw], waits[-maxw:]
                    ins.sync_info = mybir.SyncInfo(
                        on_wait=keep, on_update=list(si.on_update or [])
                    )
                    for j in range(0, len(extra), maxw):
                        count += 1
                        nop = mybir.InstNoOp(
                            name=f"ant-waitsplit-{count}", ins=[], outs=[]
                        )
                        nop.engine = ins.engine
                        nop.sync_info = mybir.SyncInfo(
                            on_wait=extra[j : j + maxw], on_update=[]
                        )
                        out.append(nop)
                out.append(ins)
            bb.instructions = out
    return count


def _dedupe_ldweights(nc):
    """Replace back-to-back InstLdweights with identical source APs by NOPs.

    The PE array keeps the stationary operand across matmuls; reloading the
    same weights between MMs only costs time. The NOP inherits the LDW's
    sync_info so queue blocking/semaphore semantics are unchanged.
    """
    pe = mybir.EngineType.PE
    count = 0
    for fn in nc.m.functions:
        for bb in fn.blocks:
            last_ldw_key = None
            out = []
            for ins in bb.instructions:
                ty = type(ins).__name__
                if getattr(ins, "engine", None) == pe:
                    if ty == "InstLdweights":
                        key = repr(ins.ins[0])
                        if key == last_ldw_key:
                            count += 1
                            nop = mybir.InstNoOp(
                                name=f"ant-ldwdedupe-{count}", ins=[], outs=[]
                            )
                            nop.engine = ins.engine
                            nop.sync_info = ins.sync_info
                            out.append(nop)
                            continue
                        last_ldw_key = key
                    elif ty in ("InstMatmult", "InstNoOp", "InstEventSemaphore"):
                        pass  # no effect on the PE weight registers
                    else:
                        last_ldw_key = None
                # non-PE instructions don't touch the PE weight registers
                out.append(ins)
            bb.instructions = out
    return count


C, B, S, D, O = 4, 8, 8192, 256, 256
NCORES = 8
N = B * S            # tokens per chunk (65536)
T = N // NCORES      # tokens per chunk per core (8192)

WS_SHIFT = 10           # weights pre-scaled by 2^10 to stay f16-normal
DEQUANT = float(2.0 ** -WS_SHIFT)

TG = 2048               # tokens per tile-group
N_TG = T // TG          # 4 tile-groups per chunk per core
MM_N = 512              # moving-operand width per matmul (PSUM one-bank limit)
PS_FD = 1024            # PSUM tile free dim (2 banks; copies read both)


def _build_program(scales):
    """Build the SPMD Bass program (same program on all cores).

    Inputs (per core): xt [C, D, T] f16, ws16 [C, D, O] f16.
    Output: out [C, 2, 128, T] f16  (o = og*128 + p, value = true_out).
    """
    f16 = mybir.dt.float16
    f32 = mybir.dt.float32
    i16 = mybir.dt.int16
    alu = mybir.AluOpType

    assert TG % PS_FD == 0 and PS_FD % MM_N == 0
    n_ps = TG // PS_FD         # PSUM tiles per og per tile-group
    n_mm = PS_FD // MM_N       # matmuls per PSUM tile per dk

    nc = bass.Bass()
    xt = nc.declare_dram_parameter("xt", [C, D, T], f16, isOutput=False)
    # ws16 is host-permuted to the SBUF-resident layout [p, (c dk o)] so the
    # load is one fully-contiguous 4KB-run-per-partition HWDGE transfer.
    ws16 = nc.declare_dram_parameter("ws16", [128, 2 * C * O], f16, isOutput=False)
    out = nc.declare_dram_parameter("out", [C, 2, 128, T], f16, isOutput=True)

    with tile.TileContext(nc) as tc:
        with (
            tc.tile_pool(name="wpool", bufs=1) as wpool,
            tc.tile_pool(name="xpool", bufs=8) as xpool,
            tc.tile_pool(name="tpool", bufs=3) as tpool,
            tc.tile_pool(name="qpool", bufs=4) as qpool,
            tc.tile_pool(name="opool", bufs=5) as opool,
            tc.tile_pool(name="ppool", bufs=4, space=bass.MemorySpace.PSUM) as ppool,
        ):
            # Resident weights: wt[c, dk, og] = [128 d, 128 o] f16.
            w_tile = wpool.tile([128, 2 * C * O], f16, tag="w")
            nc.scalar.dma_start(out=w_tile[:], in_=ws16[:])
            wt = {}
            for c in range(C):
                for dk in range(2):
                    for og in range(2):
                        base = (c * 2 + dk) * O + og * 128
                        wt[c, dk, og] = w_tile[:, base : base + 128]

            copy_rr = 0  # round-robin over the copy engines
            for c in range(C):
                inv_s = float(np.float32(1.0) / np.float32(scales[c]))
                for tg in range(N_TG):
                    # Load x tile: [p = d%128, (dk, t)]
                    x_tile = xpool.tile([128, 2 * TG], f16, tag="x")
                    src = xt[c].rearrange("(dk p) t -> p dk t", p=128)[
                        :, :, tg * TG : (tg + 1) * TG
                    ]
                    nc.sync.dma_start(
                        out=x_tile[:].rearrange("p (dk t) -> p dk t", dk=2),
                        in_=src,
                    )

                    # t16 = rne(x * inv_s) via the HW convert-on-write (RNE).
                    t16 = tpool.tile([128, 2 * TG], i16, tag="t16")
                    nc.vector.tensor_scalar(
                        t16[:], x_tile[:], inv_s, None, alu.mult
                    )
                    # qi = clip(t16, -128, 127) as f16 (exact integers)
                    qi = qpool.tile([128, 2 * TG], f16, tag="qi")
                    nc.vector.tensor_scalar(
                        qi[:], t16[:], -128.0, 127.0, alu.max, alu.min
                    )

                    # Matmuls, weight-stationary: ps [128 o, PS_FD t]
                    stage = opool.tile([128, 2 * TG], f16, tag="stage")
                    for og in range(2):
                        pss = [
                            ppool.tile([128, PS_FD], f32, name="ps", tag="ps")
                            for _ in range(n_ps)
                        ]
                        for dk in range(2):
                            for ips, ps in enumerate(pss):
                                for j in range(n_mm):
                                    t0 = ips * PS_FD + j * MM_N
                                    nc.tensor.matmul(
                                        ps[:, j * MM_N : (j + 1) * MM_N],
                                        wt[c, dk, og],
                                        qi[:, dk * TG + t0 : dk * TG + t0 + MM_N],
                                        start=(dk == 0),
                                        stop=(dk == 1),
                                    )
                        # PSUM -> SBUF staging with 2^-10 dequant folded in.
                        for ips, ps in enumerate(pss):
                            dst = stage[
                                :, og * TG + ips * PS_FD : og * TG + (ips + 1) * PS_FD
                            ]
                            # 3 ACT : 1 DVE balances measured engine busy
                            if copy_rr % 4 < 3:
                                nc.scalar.mul(dst, ps[:], DEQUANT)
                            else:
                                nc.vector.tensor_scalar(
                                    dst, ps[:], DEQUANT, None, alu.mult
                                )
                            copy_rr += 1

                    # Store: stage [p, (og t)] -> out[c, og, p, tg*TG:(tg+1)*TG]
                    # One 1MB SWDGE DMA per tile-group (big stores keep the
                    # SWDGE emission rate well above line rate); the last
                    # chunk's halves drain over the by-then-idle sync +
                    # scalar HWDGE rings in parallel with Pool's stores.
                    if c < C - 1:
                        nc.gpsimd.dma_start(
                            out=out[c]
                            .rearrange("og p t -> p og t")[
                                :, :, tg * TG : (tg + 1) * TG
                            ],
                            in_=stage[:].rearrange("p (og t) -> p og t", og=2),
                        )
                    else:
                        for og, eng in ((0, nc.sync), (1, nc.scalar)):
                            eng.dma_start(
                                out=out[c, og][:, tg * TG : (tg + 1) * TG],
                                in_=stage[:, og * TG : (og + 1) * TG],
                            )
    return nc


def _prep_inputs(x, w, scales, ncores=NCORES):
    x = np.ascontiguousarray(np.asarray(x, dtype=np.float32)).reshape(C, N, D)
    w = np.asarray(w, dtype=np.float32)
    s = np.asarray(scales, dtype=np.float32).reshape(C, 1, 1)

    ws = s * w                                            # [C, O, D] f32
    wsT = ws.transpose(0, 2, 1)                           # [C, D, O]
    ws16 = (wsT * np.float32(2.0**WS_SHIFT)).astype(np.float16)
    # Permute to the SBUF-resident layout [p, (c, dk, o)] (d = dk*128 + p)
    ws16 = np.ascontiguousarray(
        ws16.reshape(C, 2, 128, O).transpose(2, 0, 1, 3).reshape(128, 2 * C * O)
    )

    in_maps = []
    for i in range(ncores):
        xs = x[:, i * T : (i + 1) * T, :]                 # [C, T, D] view
        xtp = np.ascontiguousarray(
            xs.transpose(0, 2, 1).astype(np.float16)      # [C, D, T] f16
        )
        in_maps.append({"xt": xtp, "ws16": ws16})
    return in_maps


def run(x, w, scales, trace=False, **spmd_kwargs):
    """Compile + run on 8 cores. Returns (out, BassKernelResults)."""
    scales = np.asarray(scales, dtype=np.float32)
    nc = _build_program(scales)
    _dedupe_ldweights(nc)
    _split_sync_waits(nc)  # HW-only fixup (CoreSim chokes on raw-BIR NoOps)
    in_maps = _prep_inputs(x, w, scales)
    res = run_bass_kernel_spmd(
        nc, in_maps, core_ids=list(range(NCORES)), trace=trace, **spmd_kwargs
    )
    # Un-permute each shard: [C, 2, 128, T] (c, og, oj, t) -> [C, T, O]
    shards = [
        r["out"].reshape(C, O, T).transpose(0, 2, 1).astype(np.float32)
        for r in res.results
    ]
    out = np.concatenate(shards, axis=1)                  # [C, N, O]
    return np.ascontiguousarray(out).reshape(C, B, S, O), res


def kernel(x, w, scales):
    try:
        out, _ = run(x, w, scales, trace=False)
    except Exception:
        # One retry: the runtime occasionally reports a transient
        # NRT_EXEC_UNIT_UNRECOVERABLE on an otherwise-healthy device.
        out, _ = run(x, w, scales, trace=False)
    return out


# revision 25
# speedup vs baseline: 1.0987x; 1.0987x over previous
"""Bass/Trainium2 kernel for per-chunk fake-quant + linear.

reference semantics (per chunk c):
    q  = clip(round(x/s_c), -128, 127) * s_c
    out[c] = q @ w[c].T          # [B,S,O]

Strategy (16-bit I/O halves HBM traffic to ~32.5MB/core; measured ~97us
on HW vs 207-229us for the f32-I/O baseline; DMA saturates ~420 GB/s
wall-to-wall mid-kernel, which is the pole):
  - Data-parallel over tokens: each of 8 cores gets T = B*S/8 = 8192 tokens
    (all 4 chunks), weights replicated.
  - x is staged host-side as f16 [C, D, T] (transposed so the contraction
    dim d sits on SBUF partitions). f16 keeps 11 mantissa bits: the induced
    quant-decision flips add ~0.2% rel error vs the 2e-2 tolerance.
  - Device quantization in two DVE tensor_scalar passes (16-bit dtypes ->
    4x perf mode, ~0.3ns/elt):
      t16 = rne(x * 1/s)   as int16 (HW convert-on-write rounds RNE,
                            matching jnp.round; values in +-600)
      qi  = clip(t16, -128, 127) as f16 (small integers, exact in f16)
  - Weight-stationary matmuls: lhsT = ws16[c,dk,og] (128d x 128o) loaded
    once per 4 MMs, rhs = qi (128d x 512t), accumulated over dk into a
    PSUM tile [128o, 1024t] spanning 2 banks.  ws16 = (s*w).T * 2^10 as
    f16 (pre-scale keeps all weights f16-normal), the 2^-10 dequant is
    folded into the PSUM->SBUF copy scale.
  - PSUM->SBUF copies at FD=1024 (PSUM reads are 1x rate; 2-bank reads
    halve the per-op overhead), split 3 ACT : 1 DVE to balance engines
    (~55us busy each, under the ~80us DMA pole).
  - Output staged f16 [128o, (og t)] and DMA'd out as f16, one 1MB SWDGE
    store per tile-group (big stores keep the drain-phase store rate
    high); the last chunk's stores go out on the by-then-idle sync +
    scalar HWDGE rings to shorten the tail. Host upcasts to f32.
  - DMA rings: x loads on sync HWDGE (1MB tiles, 4KB runs/partition),
    weights + stores on gpsimd SWDGE.
  - Post-passes on the BIR: redundant back-to-back LDWEIGHTS with
    identical source APs are replaced by sync-preserving NOPs (weights
    stay resident in the PE array across the dk-inner MM pairs); excess
    per-instruction sem waits hoisted onto NOPs (walrus rejects >2).
"""

import numpy as np

import concourse.bass as bass
import concourse.tile as tile
import concourse.mybir as mybir
from concourse.bass_utils import run_bass_kernel_spmd


def _split_sync_waits(nc):
    """Hoist excess per-instruction sem waits onto preceding same-engine NOPs.

    This walrus build rejects instructions carrying >2 sync waits ("Too many
    sync wait commands", CoreV2/V3GenImpl setupSyncWait). A NOP on the same
    engine immediately before the instruction blocks the queue identically,
    so semantics are preserved.
    """
    count = 0
    for fn in nc.m.functions:
        for bb in fn.blocks:
            out = []
            for ins in bb.instructions:
                si = ins.sync_info
                waits = list(si.on_wait) if (si and si.on_wait) else []
                maxw = 1
                if len(waits) > maxw:
                    extra, keep = waits[:-maxw], waits[-maxw:]
                    ins.sync_info = mybir.SyncInfo(
                        on_wait=keep, on_update=list(si.on_update or [])
                    )
                    for j in range(0, len(extra), maxw):
                        count += 1
                        nop = mybir.InstNoOp(
                            name=f"ant-waitsplit-{count}", ins=[], outs=[]
                        )
                        nop.engine = ins.engine
                        nop.sync_info = mybir.SyncInfo(
                            on_wait=extra[j : j + maxw], on_update=[]
                        )
                        out.append(nop)
                out.append(ins)
            bb.instructions = out
    return count


def _dedupe_ldweights(nc):
    """Replace back-to-back InstLdweights with identical source APs by NOPs.

    The PE array keeps the stationary operand across matmuls; reloading the
    same weights between MMs only costs time. The NOP inherits the LDW's
    sync_info so queue blocking/semaphore semantics are unchanged.
    """
    pe = mybir.EngineType.PE
    count = 0
    for fn in nc.m.functions:
        for bb in fn.blocks:
            last_ldw_key = None
            out = []
            for ins in bb.instructions:
                ty = type(ins).__name__
                if getattr(ins, "engine", None) == pe:
                    if ty == "InstLdweights":
                        key = repr(ins.ins[0])
                        if key == last_ldw_key:
                            count += 1
                            nop = mybir.InstNoOp(
                                name=f"ant-ldwdedupe-{count}", ins=[], outs=[]
                            )
                            nop.engine = ins.engine
                            nop.sync_info = ins.sync_info
                            out.append(nop)
                            continue
                        last_ldw_key = key
                    elif ty in ("InstMatmult", "InstNoOp", "InstEventSemaphore"):
                        pass  # no effect on the PE weight registers
                    else:
                        last_ldw_key = None
                # non-PE instructions don't touch the PE weight registers
                out.append(ins)
            bb.instructions = out
    return count


C, B, S, D, O = 4, 8, 8192, 256, 256
NCORES = 8
N = B * S            # tokens per chunk (65536)
T = N // NCORES      # tokens per chunk per core (8192)

WS_SHIFT = 10           # weights pre-scaled by 2^10 to stay f16-normal
DEQUANT = float(2.0 ** -WS_SHIFT)

TG = 2048               # tokens per tile-group
N_TG = T // TG          # 4 tile-groups per chunk per core
MM_N = 512              # moving-operand width per matmul (PSUM one-bank limit)
PS_FD = 1024            # PSUM tile free dim (2 banks; copies read both)


def _build_program(scales):
    """Build the SPMD Bass program (same program on all cores).

    Inputs (per core): xt [C, D, T] f16, ws16 [C, D, O] f16.
    Output: out [C, 2, 128, T] f16  (o = og*128 + p, value = true_out).
    """
    f16 = mybir.dt.float16
    f32 = mybir.dt.float32
    i16 = mybir.dt.int16
    alu = mybir.AluOpType

    assert TG % PS_FD == 0 and PS_FD % MM_N == 0
    n_ps = TG // PS_FD         # PSUM tiles per og per tile-group
    n_mm = PS_FD // MM_N       # matmuls per PSUM tile per dk

    nc = bass.Bass()
    xt = nc.declare_dram_parameter("xt", [C, D, T], f16, isOutput=False)
    # ws16 is host-permuted to the SBUF-resident layout [p, (c dk o)] so the
    # load is one fully-contiguous 4KB-run-per-partition HWDGE transfer.
    ws16 = nc.declare_dram_parameter("ws16", [128, 2 * C * O], f16, isOutput=False)
    out = nc.declare_dram_parameter("out", [C, 2, 128, T], f16, isOutput=True)

    with tile.TileContext(nc) as tc:
        with (
            tc.tile_pool(name="wpool", bufs=1) as wpool,
            tc.tile_pool(name="xpool", bufs=8) as xpool,
            tc.tile_pool(name="tpool", bufs=3) as tpool,
            tc.tile_pool(name="qpool", bufs=4) as qpool,
            tc.tile_pool(name="opool", bufs=5) as opool,
            tc.tile_pool(name="ppool", bufs=4, space=bass.MemorySpace.PSUM) as ppool,
        ):
            # Resident weights: wt[c, dk, og] = [128 d, 128 o] f16.
            w_tile = wpool.tile([128, 2 * C * O], f16, tag="w")
            nc.scalar.dma_start(out=w_tile[:], in_=ws16[:])
            wt = {}
            for c in range(C):
                for dk in range(2):
                    for og in range(2):
                        base = (c * 2 + dk) * O + og * 128
                        wt[c, dk, og] = w_tile[:, base : base + 128]

            copy_rr = 0  # round-robin over the copy engines
            for c in range(C):
                inv_s = float(np.float32(1.0) / np.float32(scales[c]))
                for tg in range(N_TG):
                    # Load x tile: [p = d%128, (dk, t)]
                    x_tile = xpool.tile([128, 2 * TG], f16, tag="x")
                    src = xt[c].rearrange("(dk p) t -> p dk t", p=128)[
                        :, :, tg * TG : (tg + 1) * TG
                    ]
                    nc.sync.dma_start(
                        out=x_tile[:].rearrange("p (dk t) -> p dk t", dk=2),
                        in_=src,
                    )

                    # t16 = rne(x * inv_s) via the HW convert-on-write (RNE).
                    t16 = tpool.tile([128, 2 * TG], i16, tag="t16")
                    nc.vector.tensor_scalar(
                        t16[:], x_tile[:], inv_s, None, alu.mult
                    )
                    # qi = clip(t16, -128, 127) as f16 (exact integers)
                    qi = qpool.tile([128, 2 * TG], f16, tag="qi")
                    nc.vector.tensor_scalar(
                        qi[:], t16[:], -128.0, 127.0, alu.max, alu.min
                    )

                    # Matmuls, weight-stationary: ps [128 o, PS_FD t]
                    stage = opool.tile([128, 2 * TG], f16, tag="stage")
                    for og in range(2):
                        pss = [
                            ppool.tile([128, PS_FD], f32, name="ps", tag="ps")
                            for _ in range(n_ps)
                        ]
                        for dk in range(2):
                            for ips, ps in enumerate(pss):
                                for j in range(n_mm):
                                    t0 = ips * PS_FD + j * MM_N
                                    nc.tensor.matmul(
                                        ps[:, j * MM_N : (j + 1) * MM_N],
                                        wt[c, dk, og],
                                        qi[:, dk * TG + t0 : dk * TG + t0 + MM_N],
                                        start=(dk == 0),
                                        stop=(dk == 1),
                                    )
                        # PSUM -> SBUF staging with 2^-10 dequant folded in.
                        for ips, ps in enumerate(pss):
                            dst = stage[
                                :, og * TG + ips * PS_FD : og * TG + (ips + 1) * PS_FD
                            ]
                            # 3 ACT : 1 DVE balances measured engine busy
                            # mid-stream (DVE also runs the quantize passes);
                            # the last chunk flips DVE-heavy since no pass
                            # work remains during the store drain.
                            act_copy = (
                                copy_rr % 4 < 3 if c < C - 1 else copy_rr % 4 < 1
                            )
                            if act_copy:
                                nc.scalar.mul(dst, ps[:], DEQUANT)
                            else:
                                nc.vector.tensor_scalar(
                                    dst, ps[:], DEQUANT, None, alu.mult
                                )
                            copy_rr += 1

                    # Store: stage [p, (og t)] -> out[c, og, p, tg*TG:(tg+1)*TG]
                    # One 1MB SWDGE DMA per tile-group (big stores keep the
                    # SWDGE emission rate well above line rate); the last
                    # chunk's halves drain over the by-then-idle sync +
                    # scalar HWDGE rings in parallel with Pool's stores.
                    if c < C - 1:
                        nc.gpsimd.dma_start(
                            out=out[c]
                            .rearrange("og p t -> p og t")[
                                :, :, tg * TG : (tg + 1) * TG
                            ],
                            in_=stage[:].rearrange("p (og t) -> p og t", og=2),
                        )
                    else:
                        for og, eng in ((0, nc.sync), (1, nc.scalar)):
                            eng.dma_start(
                                out=out[c, og][:, tg * TG : (tg + 1) * TG],
                                in_=stage[:, og * TG : (og + 1) * TG],
                            )
    return nc


def _prep_inputs(x, w, scales, ncores=NCORES):
    x = np.ascontiguousarray(np.asarray(x, dtype=np.float32)).reshape(C, N, D)
    w = np.asarray(w, dtype=np.float32)
    s = np.asarray(scales, dtype=np.float32).reshape(C, 1, 1)

    ws = s * w                                            # [C, O, D] f32
    wsT = ws.transpose(0, 2, 1)                           # [C, D, O]
    ws16 = (wsT * np.float32(2.0**WS_SHIFT)).astype(np.float16)
    # Permute to the SBUF-resident layout [p, (c, dk, o)] (d = dk*128 + p)
    ws16 = np.ascontiguousarray(
        ws16.reshape(C, 2, 128, O).transpose(2, 0, 1, 3).reshape(128, 2 * C * O)
    )

    in_maps = []
    for i in range(ncores):
        xs = x[:, i * T : (i + 1) * T, :]                 # [C, T, D] view
        xtp = np.ascontiguousarray(
            xs.transpose(0, 2, 1).astype(np.float16)      # [C, D, T] f16
        )
        in_maps.append({"xt": xtp, "ws16": ws16})
    return in_maps


def run(x, w, scales, trace=False, **spmd_kwargs):
    """Compile + run on 8 cores. Returns (out, BassKernelResults)."""
    scales = np.asarray(scales, dtype=np.float32)
    nc = _build_program(scales)
    _dedupe_ldweights(nc)
    _split_sync_waits(nc)  # HW-only fixup (CoreSim chokes on raw-BIR NoOps)
    in_maps = _prep_inputs(x, w, scales)
    res = run_bass_kernel_spmd(
        nc, in_maps, core_ids=list(range(NCORES)), trace=trace, **spmd_kwargs
    )
    # Un-permute each shard: [C, 2, 128, T] (c, og, oj, t) -> [C, T, O]
    shards = [
        r["out"].reshape(C, O, T).transpose(0, 2, 1).astype(np.float32)
        for r in res.results
    ]
    out = np.concatenate(shards, axis=1)                  # [C, N, O]
    return np.ascontiguousarray(out).reshape(C, B, S, O), res


def kernel(x, w, scales):
    try:
        out, _ = run(x, w, scales, trace=False)
    except Exception:
        # One retry: the runtime occasionally reports a transient
        # NRT_EXEC_UNIT_UNRECOVERABLE on an otherwise-healthy device.
        out, _ = run(x, w, scales, trace=False)
    return out
